# revision 1
# baseline (speedup 1.0000x reference)
"""Trainium2 Bass kernel: 12-head attention (B=2, N=2048, C=768) on 8 NeuronCores.

Sharding: core c -> batch b = c // 4, head-group g = c % 4 (heads 3g..3g+2).
Per core: column-sharded QKV projection, head-sharded attention, 8-core
AllToAll to re-shard from (channels, all tokens) to (all channels, my token
slice), then the output projection on the core's 512-token slice (both batch
halves are projected; the host keeps the correct one).

Device layouts are channel-major ([C, tokens]) so the exp mask bias is a
per-partition (key) ACT bias and the gathered tensor feeds the projection
directly as matmul rhs. The softmax denominator comes from an extra all-ones
column appended to V (one PV matmul yields values + row sums); division is
reciprocal_approx_fast on a gpsimd partition-broadcast of the sums row.

The query axis is processed in two parity halves (tokens {even 256-blocks},
then {odd 256-blocks}); each half ends in its own AllToAll carrying the
first/second 256 tokens of every receiver's slice, so collective #1 and the
first projection pass overlap with the second half's attention compute.
"""

import numpy as np
import ml_dtypes

B, N, C = 2, 2048, 768
H, HD = 12, 64
HPG = 3            # heads per core
GPB = 4            # cores (head-groups) per batch
NCORES = 8
SCALE = float(HD) ** -0.5
MASK_NEG = -50.0
KCH = N // 128     # 16 key chunks
DCH = C // 128     # 6 contraction chunks
NH = N // 2        # 1024 tokens per parity half

bf = ml_dtypes.bfloat16

_cache = {}


def _build():
    import concourse.mybir as mybir
    import concourse.tile as tile
    from concourse import bacc

    fp32 = mybir.dt.float32
    bfl = mybir.dt.bfloat16
    EXP = mybir.ActivationFunctionType.Exp
    MULT = mybir.AluOpType.mult

    nc = bacc.Bacc(None, num_devices=NCORES)
    xT = nc.declare_dram_parameter("xT", [C, N], bfl, isOutput=False)
    wqk = nc.declare_dram_parameter("wqk", [C, 2 * HPG * HD], bfl, isOutput=False)
    wv = nc.declare_dram_parameter("wv", [C, HPG * HD], bfl, isOutput=False)
    wp = nc.declare_dram_parameter("wp", [C, C], bfl, isOutput=False)
    bp = nc.declare_dram_parameter("bp", [128, DCH], fp32, isOutput=False)
    mb = nc.declare_dram_parameter("mb", [128, KCH], fp32, isOutput=False)
    mf = nc.declare_dram_parameter("mf", [128, KCH], fp32, isOutput=False)
    out = nc.declare_dram_parameter("out", [C, 2 * 512], fp32, isOutput=True)

    with tile.TileContext(nc) as tc:
        with (
            tc.tile_pool(name="const", bufs=1) as cpool,
            tc.tile_pool(name="work", bufs=1) as wpool,
            tc.tile_pool(name="pp", bufs=5) as ppool,
        ):
            # ---------------- input loads (order = need order) ----------------
            mb_sb = cpool.tile([128, KCH], fp32, tag="mb")
            nc.sync.dma_start(mb_sb[:], mb[:])
            mf_sb = cpool.tile([128, KCH], fp32, tag="mf")
            nc.sync.dma_start(mf_sb[:], mf[:])
            wv_sb = cpool.tile([128, DCH, HPG * HD], bfl, tag="wv")
            nc.sync.dma_start(wv_sb[:], wv.rearrange("(o p) c -> p o c", p=128))
            wqk_sb = cpool.tile([128, DCH, 2 * HPG * HD], bfl, tag="wqk")
            nc.sync.dma_start(wqk_sb[:], wqk.rearrange("(o p) c -> p o c", p=128))
            xT_sb = cpool.tile([128, DCH, N], bfl, tag="xT")
            xT_r = xT.rearrange("(o p) t -> p o t", p=128)
            for tq in range(4):
                nc.sync.dma_start(
                    xT_sb[:, :, tq * 512 : (tq + 1) * 512],
                    xT_r[:, :, tq * 512 : (tq + 1) * 512],
                )
            wp_sb = cpool.tile([128, DCH, C], bfl, tag="wp")
            nc.sync.dma_start(wp_sb[:], wp.rearrange("(o p) c -> p o c", p=128))
            bp_sb = cpool.tile([128, DCH], fp32, tag="bp")
            nc.sync.dma_start(bp_sb[:], bp[:])

            # preload the exp table set while DMAs run
            warm = cpool.tile([1, 8], fp32, tag="warm")
            nc.vector.memset(warm[:], 0.0)
            nc.scalar.activation(warm[:], warm[:], EXP)

            qT = wpool.tile([64, HPG, N], bfl, tag="qT")
            kT = wpool.tile([64, HPG, N], bfl, tag="kT")
            V3 = wpool.tile([128, KCH, HPG, HD + 1], bfl, tag="V3")
            # wqk col order: [q_h0 k_h0 q_h1 k_h1 q_h2 k_h2]
            dests = [(qT, 0), (kT, 0), (qT, 1), (kT, 1), (qT, 2), (kT, 2)]

            def qk_mtile(qkps, m, quarters):
                for tq in quarters:
                    qk_t = qkps.tile([128, 512], fp32, tag="qk")
                    for kk in range(DCH):
                        nc.tensor.matmul(
                            qk_t[:],
                            lhsT=wqk_sb[:, kk, m * 128 : (m + 1) * 128],
                            rhs=xT_sb[:, kk, tq * 512 : (tq + 1) * 512],
                            start=(kk == 0),
                            stop=(kk == DCH - 1),
                        )
                    for half in range(2):
                        dtile, j = dests[m * 2 + half]
                        nc.vector.tensor_copy(
                            dtile[:, j, tq * 512 : (tq + 1) * 512],
                            qk_t[half * 64 : (half + 1) * 64, :],
                        )

            # ---- V projection + first QK Mtile, interleaved with xT arrival ----
            aux_cm = tc.tile_pool(name="aux", bufs=2, space="PSUM")
            aux = aux_cm.__enter__()
            qkps = aux
            for tq in range(4):
                for m in range(3):
                    qk_mtile(qkps, m, [tq])
                for i in range(4 * tq, 4 * tq + 4):
                    v_t = aux.tile([128, 512], fp32, tag="qk", name="v_t")[
                        :, : HPG * HD
                    ]
                    for kk in range(DCH):
                        nc.tensor.matmul(
                            v_t[:],
                            lhsT=xT_sb[:, kk, i * 128 : (i + 1) * 128],
                            rhs=wv_sb[:, kk, :],
                            start=(kk == 0),
                            stop=(kk == DCH - 1),
                        )
                    nc.vector.tensor_scalar_mul(
                        V3[:, i, :, 0:HD],
                        v_t[:].rearrange("p (h d) -> p h d", h=HPG),
                        mf_sb[:, i : i + 1],
                    )
                    nc.vector.tensor_copy(
                        V3[:, i, :, HD],
                        mf_sb[:, i : i + 1].to_broadcast((128, HPG)),
                    )

            # ---------------- attention (parity halves) ----------------
            sps_cm = tc.tile_pool(name="sps", bufs=2, space="PSUM")
            sps = sps_cm.__enter__()
            ops_cm = tc.tile_pool(name="ops", bufs=1, space="PSUM")
            ops = ops_cm.__enter__()

            OnA = [wpool.tile([128, NH], bfl, tag=f"OnA{q}", name=f"OnA{q}") for q in range(2)]
            OnB = [wpool.tile([64, NH], bfl, tag=f"OnB{q}", name=f"OnB{q}") for q in range(2)]

            def attention_head(h, qh, extra=None):
                q_rl = qT[0:64, h, :].rearrange("p (k par c) -> p par k c", par=2, c=256)
                o_t = ops.tile([HD + 1, NH], fp32, tag="o")
                for i in range(KCH):
                    if extra is not None and i % 2 == 0 and i // 2 < len(extra):
                        extra[i // 2]()
                    s_t = sps.tile([128, NH], fp32, tag="s")
                    for n2 in range(2):
                        nc.tensor.matmul(
                            s_t[:, n2 * 512 : (n2 + 1) * 512],
                            lhsT=kT[:, h, i * 128 : (i + 1) * 128],
                            rhs=q_rl[:, qh, 2 * n2 : 2 * n2 + 2, :],
                            start=True,
                            stop=True,
                        )
                    p_t = ppool.tile([128, NH], bfl, tag="p")
                    nc.scalar.activation(
                        p_t[:], s_t[:], EXP, bias=mb_sb[:, i : i + 1], scale=SCALE
                    )
                    for n2 in range(2):
                        nc.tensor.matmul(
                            o_t[:, n2 * 512 : (n2 + 1) * 512],
                            lhsT=V3[:, i, h, :],
                            rhs=p_t[:, n2 * 512 : (n2 + 1) * 512],
                            start=(i == 0),
                            stop=(i == KCH - 1),
                        )
                sums = wpool.tile([1, NH], fp32, tag="sums")
                nc.scalar.copy(sums[:], o_t[HD : HD + 1, :])
                o_raw = wpool.tile([HD, NH], fp32, tag=f"oraw{h}")
                nc.vector.tensor_copy(o_raw[:], o_t[0:HD, :])
                rbraw = wpool.tile([HD, NH], fp32, tag="rbraw")
                nc.gpsimd.partition_broadcast(rbraw[:], sums[:])
                rb = wpool.tile([HD, NH], fp32, tag="rb")
                nc.vector.reciprocal_approx_fast(rb[:], rbraw[:])
                dst = OnA[qh][h * 64 : (h + 1) * 64, :] if h < 2 else OnB[qh][:, :]
                return nc.vector.tensor_tensor(dst, o_raw[:], rb[:], MULT)

            def bounce_and_a2a(qh, agi, ago):
                for j in range(NCORES):
                    g = j % GPB
                    nc.sync.dma_start(
                        agi[j * 192 : j * 192 + 128, :],
                        OnA[qh][:, g * 256 : (g + 1) * 256],
                    )
                    nc.sync.dma_start(
                        agi[j * 192 + 128 : (j + 1) * 192, :],
                        OnB[qh][:, g * 256 : (g + 1) * 256],
                    )
                nc.gpsimd.collective_compute(
                    "AllToAll",
                    mybir.AluOpType.bypass,
                    replica_groups=[[0, 1, 2, 3, 4, 5, 6, 7]],
                    ins=[agi[:].opt()],
                    outs=[ago[:].opt()],
                )

            at_sb = wpool.tile([128, 2 * DCH, 512], bfl, tag="at")
            out_t = out.rearrange("(o p) t -> p o t", p=128)

            at_r = at_sb[:].rearrange("p (b k) t -> p b k t", b=2)

            def proj_pass(pjps, qh, ago, after=None):
                from concourse.bass import _add_dep_helper

                dma_i = nc.sync.dma_start(
                    at_sb[:, :, qh * 256 : (qh + 1) * 256],
                    ago.rearrange("(o p) t -> p o t", p=128),
                )
                if after is not None:
                    _add_dep_helper(
                        dma_i.ins, after.ins, sync=False, reason="late proj"
                    )
                for m in range(DCH):
                    y_ps = pjps.tile([128, 512], fp32, tag="qk", name="y_ps")
                    for kk in range(DCH):
                        mm_i = nc.tensor.matmul(
                            y_ps[:],
                            lhsT=wp_sb[:, kk, m * 128 : (m + 1) * 128],
                            rhs=at_r[:, :, kk, qh * 256 : (qh + 1) * 256],
                            start=(kk == 0),
                            stop=(kk == DCH - 1),
                        )
                        if after is not None and m == 0 and kk == 0:
                            _add_dep_helper(
                                mm_i.ins, after.ins, sync=False, reason="late proj"
                            )
                    y_sb = ppool.tile([128, 512], fp32, tag="y")
                    nc.vector.tensor_scalar_add(y_sb[:], y_ps[:], bp_sb[:, m : m + 1])
                    nc.sync.dma_start(
                        out_t[:, m, :].rearrange("p (b q) -> p b q", b=2)[
                            :, :, qh * 256 : (qh + 1) * 256
                        ],
                        y_sb[:].rearrange("p (b q) -> p b q", b=2),
                    )

            ag_in = [
                nc.dram_tensor(f"ag_in{q}", [NCORES * HPG * HD, 256], bfl)
                for q in range(2)
            ]
            ag_out = [
                nc.dram_tensor(f"ag_out{q}", [NCORES * HPG * HD, 256], bfl)
                for q in range(2)
            ]

            # ---- half 0 (even 256-token blocks); QK m=1,2 interleave between heads
            attention_head(0, 0)
            attention_head(1, 0)
            attention_head(2, 0)
            bounce_and_a2a(0, ag_in[0], ag_out[0])

            # ---- half 1 (odd blocks); A2A#0 + proj pass 0 hide under compute
            attention_head(0, 1)
            attention_head(1, 1)
            last_norm = attention_head(2, 1)
            proj_pass(aux, 0, ag_out[0], after=last_norm)
            bounce_and_a2a(1, ag_in[1], ag_out[1])
            proj_pass(aux, 1, ag_out[1])

            ops_cm.__exit__(None, None, None)
            sps_cm.__exit__(None, None, None)
            aux_cm.__exit__(None, None, None)

    nc.finalize()
    return nc


def _shard_inputs(x, mask, w_qkv, w_proj, b_proj):
    in_maps = []
    for c in range(NCORES):
        b, g = c // GPB, c % GPB
        heads = [3 * g, 3 * g + 1, 3 * g + 2]
        qk_cols = [
            base + h * HD + d for h in heads for base in (0, C) for d in range(HD)
        ]
        v_cols = [2 * C + h * HD + d for h in heads for d in range(HD)]
        mrow = mask[b].astype(np.float32)
        in_maps.append(
            {
                "xT": np.ascontiguousarray(x[b].T).astype(bf),
                "wqk": np.ascontiguousarray(w_qkv[:, qk_cols]).astype(bf),
                "wv": np.ascontiguousarray(w_qkv[:, v_cols]).astype(bf),
                "wp": w_proj.astype(bf),
                "bp": np.ascontiguousarray(
                    b_proj.astype(np.float32).reshape(DCH, 128).T
                ),
                "mb": np.ascontiguousarray(
                    np.where(mrow > 0.5, 0.0, MASK_NEG)
                    .astype(np.float32)
                    .reshape(KCH, 128)
                    .T
                ),
                "mf": np.ascontiguousarray(mrow.reshape(KCH, 128).T),
            }
        )
    return in_maps


def kernel(x, mask, w_qkv, w_proj, b_proj, _trace=False):
    from concourse.bass_utils import run_bass_kernel_spmd

    x = np.asarray(x, dtype=np.float32)
    mask = np.asarray(mask)
    w_qkv = np.asarray(w_qkv, dtype=np.float32)
    w_proj = np.asarray(w_proj, dtype=np.float32)
    b_proj = np.asarray(b_proj, dtype=np.float32)
    if "nc" not in _cache:
        _cache["nc"] = _build()
    nc = _cache["nc"]
    in_maps = _shard_inputs(x, mask, w_qkv, w_proj, b_proj)
    res = run_bass_kernel_spmd(nc, in_maps, core_ids=list(range(NCORES)), trace=_trace)
    y = np.empty((B, N, C), dtype=np.float32)
    for c in range(NCORES):
        b, g = c // GPB, c % GPB
        y[b, g * 512 : (g + 1) * 512] = np.asarray(
            res.results[c]["out"][:, b * 512 : (b + 1) * 512]
        ).T
    if _trace:
        _cache["last_exec_time_ns"] = res.exec_time_ns
        _cache["last_profile"] = res.profile_json
    return y



# revision 7
# speedup vs baseline: 1.4732x; 1.4732x over previous
"""Trainium2 Bass kernel: 12-head attention (B=2, N=2048, C=768) on 8 NeuronCores.

Sharding: core c -> batch b = c // 4, head-group g = c % 4 (heads 3g..3g+2).

Key optimizations over the naive head-sharded layout:
- Mask compaction: the key mask is host-visible, so tokens of each batch are
  permuted so kept keys (~1002/1034 of 2048) come first; K/V/attention only
  process ceil(max_kept/128)*128 keys instead of 2048 (QK, exp, PV all shrink
  ~45%). Queries still cover all 2048 tokens; the host un-permutes at the end.
  Keys beyond the kept count are real (masked) tokens whose V rows and
  softmax-denominator column are zeroed via the mf vector, so they contribute
  exactly 0 to numerator and denominator (matching the reference).
- Head packing on the PE array: wq/wk are laid out [q0|q1] / [k0|k1] so head 0
  lives on SBUF partitions 0-63 and head 1 on 64-127. QK^T has contraction 64,
  so head-0 and head-1 matmuls occupy disjoint row-halves of the 128x128 array
  (tile_position auto-derived from base partitions) and run concurrently.
  Head 2 uses duplicated columns [q2|q2] / [k2|k2] and alternates halves.
- Softmax denominator via an extra all-ones column appended to V (one PV
  matmul yields values + row sums). The per-query normalization runs on
  DVE + gpsimd only (copy, partition_broadcast, reciprocal_approx_fast,
  multiply) keeping the scalar engine dedicated to exp.
- Fully-useful 8-way AllToAll: each parity-half's 1024 query tokens are split
  into 8 blocks of 128; receiver j gets all 768 channels of token-block j for
  BOTH batches (4 sender cores per batch), so the projection runs on 256
  columns (128 tokens x 2 batches) per half with zero wasted wire bytes or
  flops (the naive 8-way reshard ships/projects 2x and discards half).
  Queries are processed in two parity halves so the first A2A and proj
  overlap with the second half's attention compute.
"""

import numpy as np
import ml_dtypes

B, N, C = 2, 2048, 768
H, HD = 12, 64
HPG = 3            # heads per core
GPB = 4            # cores (head-groups) per batch
NCORES = 8
SCALE = float(HD) ** -0.5
DCH = C // 128     # 6 contraction chunks
NH = N // 2        # 1024 query tokens per parity half

bf = ml_dtypes.bfloat16

_cache = {}


def _build(nkch):
    import concourse.mybir as mybir
    import concourse.tile as tile
    from concourse import bacc

    fp32 = mybir.dt.float32
    bfl = mybir.dt.bfloat16
    EXP = mybir.ActivationFunctionType.Exp
    MULT = mybir.AluOpType.mult

    NK = nkch * 128  # padded key count

    nc = bacc.Bacc(None, num_devices=NCORES)
    xT = nc.declare_dram_parameter("xT", [C, N], bfl, isOutput=False)
    wq = nc.declare_dram_parameter("wq", [C, 256], bfl, isOutput=False)
    wk = nc.declare_dram_parameter("wk", [C, 256], bfl, isOutput=False)
    wv = nc.declare_dram_parameter("wv", [C, HPG * HD], bfl, isOutput=False)
    wp = nc.declare_dram_parameter("wp", [C, C], bfl, isOutput=False)
    bp = nc.declare_dram_parameter("bp", [128, DCH], fp32, isOutput=False)
    mf = nc.declare_dram_parameter("mf", [128, nkch], fp32, isOutput=False)
    out = nc.declare_dram_parameter("out", [C, 512], fp32, isOutput=True)

    with tile.TileContext(nc) as tc:
        with (
            tc.tile_pool(name="const", bufs=1) as cpool,
            tc.tile_pool(name="work", bufs=1) as wpool,
            tc.tile_pool(name="pp", bufs=2) as ppool,
        ):
            # ---------------- input loads (order = need order) ----------------
            mf_sb = cpool.tile([128, nkch], fp32, tag="mf")
            nc.sync.dma_start(mf_sb[:], mf[:])
            wq_sb = cpool.tile([128, DCH, 256], bfl, tag="wq")
            nc.sync.dma_start(wq_sb[:], wq.rearrange("(o p) c -> p o c", p=128))
            wk_sb = cpool.tile([128, DCH, 256], bfl, tag="wk")
            nc.sync.dma_start(wk_sb[:], wk.rearrange("(o p) c -> p o c", p=128))
            xT_sb = cpool.tile([128, DCH, N], bfl, tag="xT")
            xT_r = xT.rearrange("(o p) t -> p o t", p=128)
            for tq in range(4):
                nc.sync.dma_start(
                    xT_sb[:, :, tq * 512 : (tq + 1) * 512],
                    xT_r[:, :, tq * 512 : (tq + 1) * 512],
                )
            wv_sb = cpool.tile([128, DCH, HPG * HD], bfl, tag="wv")
            nc.sync.dma_start(wv_sb[:], wv.rearrange("(o p) c -> p o c", p=128))
            wp_sb = cpool.tile([128, DCH, C], bfl, tag="wp")
            nc.sync.dma_start(wp_sb[:], wp.rearrange("(o p) c -> p o c", p=128))
            bp_sb = cpool.tile([128, DCH], fp32, tag="bp")
            nc.sync.dma_start(bp_sb[:], bp[:])

            # preload the exp table set while DMAs run
            warm = cpool.tile([1, 8], fp32, tag="warm")
            nc.vector.memset(warm[:], 0.0)
            nc.scalar.activation(warm[:], warm[:], EXP)

            qs = wpool.tile([128, N], bfl, tag="qs")      # [q0 | q1] channel-major
            qs2 = wpool.tile([128, N], bfl, tag="qs2")    # [q2 | q2]
            kst = wpool.tile([128, 2, NK], bfl, tag="kst")  # [:,0]=[k0|k1] [:,1]=[k2|k2]
            V3 = wpool.tile([128, nkch, HPG, HD + 1], bfl, tag="V3")

            aux_cm = tc.tile_pool(name="aux", bufs=2, space="PSUM")
            aux = aux_cm.__enter__()

            # ---- QKV projections, interleaved with xT quarter arrival ----
            for tq in range(4):
                sl = slice(tq * 512, (tq + 1) * 512)
                # K projection restricted to the NK kept+pad keys
                kw = min(512, max(0, NK - tq * 512))
                for m in range(2):
                    if kw > 0:
                        k_t = aux.tile([128, 512], fp32, tag="qk", name="k_t")
                        for kk in range(DCH):
                            nc.tensor.matmul(
                                k_t[:, :kw],
                                lhsT=wk_sb[:, kk, m * 128 : (m + 1) * 128],
                                rhs=xT_sb[:, kk, tq * 512 : tq * 512 + kw],
                                start=(kk == 0),
                                stop=(kk == DCH - 1),
                            )
                        nc.vector.tensor_copy(
                            kst[:, m, tq * 512 : tq * 512 + kw], k_t[:, :kw]
                        )
                    q_t = aux.tile([128, 512], fp32, tag="qk", name="q_t")
                    for kk in range(DCH):
                        nc.tensor.matmul(
                            q_t[:],
                            lhsT=wq_sb[:, kk, m * 128 : (m + 1) * 128],
                            rhs=xT_sb[:, kk, sl],
                            start=(kk == 0),
                            stop=(kk == DCH - 1),
                        )
                    nc.vector.tensor_copy((qs if m == 0 else qs2)[:, sl], q_t[:])
                # V projection for the key chunks inside this quarter
                for c in range(4 * tq, min(4 * tq + 4, nkch)):
                    v_t = aux.tile([128, 512], fp32, tag="qk", name="v_t")[
                        :, : HPG * HD
                    ]
                    for kk in range(DCH):
                        nc.tensor.matmul(
                            v_t[:],
                            lhsT=xT_sb[:, kk, c * 128 : (c + 1) * 128],
                            rhs=wv_sb[:, kk, :],
                            start=(kk == 0),
                            stop=(kk == DCH - 1),
                        )
                    nc.vector.tensor_scalar_mul(
                        V3[:, c, :, 0:HD],
                        v_t[:].rearrange("p (h d) -> p h d", h=HPG),
                        mf_sb[:, c : c + 1],
                    )
                    nc.vector.tensor_copy(
                        V3[:, c, :, HD],
                        mf_sb[:, c : c + 1].to_broadcast((128, HPG)),
                    )

            # ---------------- attention (parity halves) ----------------
            sps_cm = tc.tile_pool(name="sps", bufs=2, space="PSUM")
            sps = sps_cm.__enter__()
            ops_cm = tc.tile_pool(name="ops", bufs=1, space="PSUM")
            ops = ops_cm.__enter__()

            OnA = [wpool.tile([128, NH], bfl, tag=f"OnA{q}", name=f"OnA{q}") for q in range(2)]
            OnB = [wpool.tile([64, NH], bfl, tag=f"OnB{q}", name=f"OnB{q}") for q in range(2)]

            def unit(h, qh):
                """Attention for head h over query parity-half qh (1024 tokens)."""
                ksrc = kst[:, 0] if h < 2 else kst[:, 1]
                qsrc = qs if h < 2 else qs2
                p_t = ppool.tile([128, nkch, NH], bfl, tag="p", name="p_t")
                o_t = ops.tile([HD + 1, NH], fp32, tag="o")
                for c in range(nkch):
                    if h == 0:
                        base = 0
                    elif h == 1:
                        base = 64
                    else:
                        base = 64 * (c % 2)
                    q_rl = qsrc[base : base + 64, :].rearrange(
                        "p (k par t) -> p par k t", par=2, t=256
                    )
                    s_t = sps.tile([128, NH], fp32, tag="s")
                    for n2 in range(2):
                        nc.tensor.matmul(
                            s_t[:, n2 * 512 : (n2 + 1) * 512],
                            lhsT=ksrc[base : base + 64, c * 128 : (c + 1) * 128],
                            rhs=q_rl[:, qh, 2 * n2 : 2 * n2 + 2, :],
                            start=True,
                            stop=True,
                        )
                    nc.scalar.activation(p_t[:, c, :], s_t[:], EXP, scale=SCALE)
                    for n2 in range(2):
                        nc.tensor.matmul(
                            o_t[:, n2 * 512 : (n2 + 1) * 512],
                            lhsT=V3[:, c, h, :],
                            rhs=p_t[:, c, n2 * 512 : (n2 + 1) * 512],
                            start=(c == 0),
                            stop=(c == nkch - 1),
                        )
                sums = wpool.tile([1, NH], fp32, tag="sums", bufs=2)
                nc.vector.tensor_copy(sums[:], o_t[HD : HD + 1, :])
                rbb = wpool.tile([HD, NH], fp32, tag="rbb", bufs=2)
                nc.gpsimd.partition_broadcast(rbb[:], sums[:])
                rb = wpool.tile([HD, NH], fp32, tag="rb", bufs=2)
                nc.vector.reciprocal_approx_fast(rb[:], rbb[:])
                dst = OnA[qh][h * 64 : (h + 1) * 64, :] if h < 2 else OnB[qh][:, :]
                return nc.vector.tensor_tensor(dst, o_t[0:HD, :], rb[:], MULT)

            ag_in = [
                nc.dram_tensor(f"ag_in{q}", [NCORES * HPG * HD, 128], bfl)
                for q in range(2)
            ]
            ag_out = [
                nc.dram_tensor(f"ag_out{q}", [NCORES * HPG * HD, 128], bfl)
                for q in range(2)
            ]

            def bounce_and_a2a(qh):
                agi, ago = ag_in[qh], ag_out[qh]
                agi_r = agi.rearrange("(j p) t -> p j t", j=NCORES)
                nc.sync.dma_start(
                    agi_r[0:128, :, :],
                    OnA[qh][:, :].rearrange("p (j t) -> p j t", j=NCORES),
                )
                nc.sync.dma_start(
                    agi_r[128:192, :, :],
                    OnB[qh][:, :].rearrange("p (j t) -> p j t", j=NCORES),
                )
                nc.gpsimd.collective_compute(
                    "AllToAll",
                    mybir.AluOpType.bypass,
                    replica_groups=[[0, 1, 2, 3, 4, 5, 6, 7]],
                    ins=[agi[:].opt()],
                    outs=[ago[:].opt()],
                )

            out_r = out.rearrange("(o p) t -> p o t", p=128)

            def proj_pass(qh):
                at_sb = wpool.tile(
                    [128, 2, DCH, 128], bfl, tag="at", bufs=2, name="at_sb"
                )
                nc.sync.dma_start(
                    at_sb[:], ag_out[qh].rearrange("(b o p) t -> p b o t", p=128, b=2)
                )
                for m in range(DCH):
                    y_ps = aux.tile([128, 512], fp32, tag="qk", name="y_ps")[:, :256]
                    for kk in range(DCH):
                        nc.tensor.matmul(
                            y_ps[:].rearrange("p (b t) -> p b t", b=2),
                            lhsT=wp_sb[:, kk, m * 128 : (m + 1) * 128],
                            rhs=at_sb[:, :, kk, :],
                            start=(kk == 0),
                            stop=(kk == DCH - 1),
                        )
                    y_sb = wpool.tile([128, 256], fp32, tag="y", bufs=2, name="y_sb")
                    nc.vector.tensor_scalar_add(y_sb[:], y_ps[:], bp_sb[:, m : m + 1])
                    nc.sync.dma_start(
                        out_r[:, m, qh * 256 : (qh + 1) * 256], y_sb[:]
                    )

            # ---- half 0 (even 256-token parity blocks)
            unit(0, 0)
            unit(1, 0)
            unit(2, 0)
            bounce_and_a2a(0)

            # ---- half 1; A2A#0 + proj pass 0 hide under its compute.
            # bounce/A2A#1 is issued before proj#0 so proj#0's matmuls run
            # (and keep the PE warm) while A2A#1 is in flight.
            unit(0, 1)
            unit(1, 1)
            unit(2, 1)
            bounce_and_a2a(1)
            proj_pass(0)
            proj_pass(1)

            ops_cm.__exit__(None, None, None)
            sps_cm.__exit__(None, None, None)
            aux_cm.__exit__(None, None, None)

    nc.finalize()
    return nc


def _prep(x, mask, w_qkv, w_proj, b_proj):
    """Host-side compaction: per-batch token permutation (kept keys first) and
    per-core input shards."""
    perms, counts = [], []
    for b in range(B):
        perm = np.argsort(1 - mask[b], kind="stable")
        perms.append(perm)
        counts.append(int(mask[b].sum()))
    nkch = max(1, int(np.ceil(max(counts) / 128)))
    NK = nkch * 128

    xTs = []
    mfs = []
    for b in range(B):
        xp = np.ascontiguousarray(x[b][perms[b]].T).astype(bf)
        xTs.append(xp)
        m = np.zeros(NK, dtype=np.float32)
        m[: counts[b]] = 1.0
        mfs.append(np.ascontiguousarray(m.reshape(nkch, 128).T))

    bp_t = np.ascontiguousarray(b_proj.astype(np.float32).reshape(DCH, 128).T)
    wp_t = w_proj.astype(bf)

    in_maps = []
    for c in range(NCORES):
        b, g = c // GPB, c % GPB
        heads = [3 * g, 3 * g + 1, 3 * g + 2]
        q_cols = [h * HD + d for h in (heads[0], heads[1], heads[2], heads[2]) for d in range(HD)]
        k_cols = [C + h * HD + d for h in (heads[0], heads[1], heads[2], heads[2]) for d in range(HD)]
        v_cols = [2 * C + h * HD + d for h in heads for d in range(HD)]
        in_maps.append(
            {
                "xT": xTs[b],
                "wq": np.ascontiguousarray(w_qkv[:, q_cols]).astype(bf),
                "wk": np.ascontiguousarray(w_qkv[:, k_cols]).astype(bf),
                "wv": np.ascontiguousarray(w_qkv[:, v_cols]).astype(bf),
                "wp": wp_t,
                "bp": bp_t,
                "mf": mfs[b],
            }
        )
    return in_maps, perms, nkch


def kernel(x, mask, w_qkv, w_proj, b_proj, _trace=False):
    from concourse.bass_utils import run_bass_kernel_spmd

    x = np.asarray(x, dtype=np.float32)
    mask = np.asarray(mask)
    w_qkv = np.asarray(w_qkv, dtype=np.float32)
    w_proj = np.asarray(w_proj, dtype=np.float32)
    b_proj = np.asarray(b_proj, dtype=np.float32)
    in_maps, perms, nkch = _prep(x, mask, w_qkv, w_proj, b_proj)
    if ("nc", nkch) not in _cache:
        _cache[("nc", nkch)] = _build(nkch)
    nc = _cache[("nc", nkch)]
    res = run_bass_kernel_spmd(nc, in_maps, core_ids=list(range(NCORES)), trace=_trace)
    y = np.empty((B, N, C), dtype=np.float32)
    for c in range(NCORES):
        k4, hf = c // 2, c % 2
        o = np.asarray(res.results[c]["out"])
        for qh in range(2):
            base = k4 * 512 + qh * 256 + hf * 128
            for b in range(B):
                y[b, perms[b][base : base + 128]] = o[
                    :, qh * 256 + b * 128 : qh * 256 + (b + 1) * 128
                ].T
    if _trace:
        _cache["last_exec_time_ns"] = res.exec_time_ns
        _cache["last_profile"] = res.profile_json
    return y


# revision 10
# speedup vs baseline: 1.6194x; 1.0992x over previous
"""Trainium2 Bass kernel: 12-head attention (B=2, N=2048, C=768) on 8 NeuronCores.

Sharding: core c -> batch b = c // 4, head-group g = c % 4 (heads 3g..3g+2).

Key optimizations over the naive head-sharded layout:

- Mask compaction: the key mask is host-visible, so tokens of each batch are
  permuted so kept keys (~1002/1034 of 2048) come first; K/V/attention only
  process ceil(max_kept/128)*128 keys instead of 2048 (QK, exp, PV all shrink
  ~45%). Queries still cover all 2048 tokens; the host un-permutes at the end.
  Keys beyond the kept count are real (masked) tokens whose V rows and
  softmax-denominator column are zeroed via the mf vector, so they contribute
  exactly 0 to numerator and denominator (matching the reference).

- Head packing on the PE array: wq/wk are laid out [q0|q1] / [k0|k1] so head 0
  lives on SBUF partitions 0-63 and head 1 on 64-127. QK^T has contraction 64,
  so head-0/head-1 matmuls occupy disjoint row-halves of the 128x128 array
  (tile_position auto-derived from base partitions) and run concurrently, and
  their LDWEIGHTS overlap the other head's in-flight matmuls. Heads 0/1 are
  processed chunk-interleaved in one "pair unit" (two PSUM o-accumulators);
  head 2 uses duplicated columns [q2|q2] / [k2|k2] and alternates halves.

- The attention phase is scalar-engine(exp)-bound, so everything else hides
  under it: the kst[:,1]/qs2/V projections run as fill-in work inserted
  between attention chunks; attention starts as soon as the k0|k1 and
  half-0 q0|q1 projections land instead of after the full QKV phase. PV
  matmuls for chunk c are emitted after chunk c+1's QK so the FIFO tensor
  queue never head-of-line blocks on the exp.

- Softmax denominator via an extra all-ones column appended to V (one PV
  matmul yields values + row sums). Normalization never touches the scalar
  engine (DVE copy + gpsimd row-copy/broadcast + DVE reciprocal/multiply),
  and the PSUM accumulator is released after a single copy so the next
  unit's PV can start immediately (keeps the PE HAM-warm).

- Fully-useful 8-way AllToAll per query half: each half's 1024 tokens split
  into 8 blocks of 128; receiver j gets all 768 channels of token-block j for
  BOTH batches, so the projection runs on 256 columns (128 tokens x 2
  batches) per half with zero wasted wire bytes or flops. A2A#0 and proj#0
  hide under the second half's attention; during the exposed A2A#1 the PE
  runs proj#0 plus warm-up filler matmuls so the final projection executes
  at full clock. proj DMAs are sequenced before the next collective because
  DMAs issued after a collective barrier-wait on it.
"""

import numpy as np
import ml_dtypes

B, N, C = 2, 2048, 768
H, HD = 12, 64
HPG = 3            # heads per core
GPB = 4            # cores (head-groups) per batch
NCORES = 8
SCALE = float(HD) ** -0.5
DCH = C // 128     # 6 contraction chunks
NH = N // 2        # 1024 query tokens per half

bf = ml_dtypes.bfloat16

_cache = {}


def _build(nkch):
    import concourse.mybir as mybir
    import concourse.tile as tile
    from concourse import bacc

    fp32 = mybir.dt.float32
    bfl = mybir.dt.bfloat16
    EXP = mybir.ActivationFunctionType.Exp
    MULT = mybir.AluOpType.mult

    NK = nkch * 128  # padded key count

    nc = bacc.Bacc(None, num_devices=NCORES)
    xT = nc.declare_dram_parameter("xT", [C, N], bfl, isOutput=False)
    wq = nc.declare_dram_parameter("wq", [C, 256], bfl, isOutput=False)
    wk = nc.declare_dram_parameter("wk", [C, 256], bfl, isOutput=False)
    wv = nc.declare_dram_parameter("wv", [C, HPG * HD], bfl, isOutput=False)
    wp = nc.declare_dram_parameter("wp", [C, C], bfl, isOutput=False)
    bp = nc.declare_dram_parameter("bp", [128, DCH], fp32, isOutput=False)
    mf = nc.declare_dram_parameter("mf", [128, nkch], fp32, isOutput=False)
    out = nc.declare_dram_parameter("out", [C, 512], fp32, isOutput=True)

    with tile.TileContext(nc) as tc:
        with (
            tc.tile_pool(name="const", bufs=1) as cpool,
            tc.tile_pool(name="work", bufs=1) as wpool,
            tc.tile_pool(name="pp", bufs=2) as ppool,
        ):
            # ---------------- input loads (order = need order) ----------------
            mf_sb = cpool.tile([128, nkch], fp32, tag="mf")
            nc.sync.dma_start(mf_sb[:], mf[:])
            wk_sb = cpool.tile([128, DCH, 256], bfl, tag="wk")
            nc.sync.dma_start(wk_sb[:], wk.rearrange("(o p) c -> p o c", p=128))
            wq_sb = cpool.tile([128, DCH, 256], bfl, tag="wq")
            nc.sync.dma_start(wq_sb[:], wq.rearrange("(o p) c -> p o c", p=128))
            xT_sb = cpool.tile([128, DCH, N], bfl, tag="xT")
            xT_r = xT.rearrange("(o p) t -> p o t", p=128)
            for tq in range(3):
                nc.sync.dma_start(
                    xT_sb[:, :, tq * 512 : (tq + 1) * 512],
                    xT_r[:, :, tq * 512 : (tq + 1) * 512],
                )
            wv_sb = cpool.tile([128, DCH, HPG * HD], bfl, tag="wv")
            nc.sync.dma_start(wv_sb[:], wv.rearrange("(o p) c -> p o c", p=128))
            nc.sync.dma_start(
                xT_sb[:, :, 3 * 512 : 4 * 512], xT_r[:, :, 3 * 512 : 4 * 512]
            )
            wp_sb = cpool.tile([128, DCH, C], bfl, tag="wp")
            nc.sync.dma_start(wp_sb[:], wp.rearrange("(o p) c -> p o c", p=128))
            bp_sb = cpool.tile([128, DCH], fp32, tag="bp")
            nc.sync.dma_start(bp_sb[:], bp[:])

            # preload the exp table set while DMAs run
            warm = cpool.tile([1, 8], fp32, tag="warm")
            nc.vector.memset(warm[:], 0.0)
            nc.scalar.activation(warm[:], warm[:], EXP)

            qs = wpool.tile([128, N], bfl, tag="qs")      # [q0 | q1] channel-major
            qs2 = wpool.tile([128, N], bfl, tag="qs2")    # [q2 | q2]
            kst = wpool.tile([128, 2, NK], bfl, tag="kst")  # [:,0]=[k0|k1] [:,1]=[k2|k2]
            V3 = wpool.tile([128, nkch, HPG, HD + 1], bfl, tag="V3")

            # PSUM: tag "s" 2 slots x 2 banks (QK scores + all projection /
            # fill-in tiles), tag "o" 2 slots x 2 banks (live PV accumulators).
            sps_cm = tc.tile_pool(name="sps", bufs=2, space="PSUM")
            sps = sps_cm.__enter__()
            ops_cm = tc.tile_pool(name="ops", bufs=2, space="PSUM")
            ops = ops_cm.__enter__()

            def qk_pass(which, m, hf):
                """Q or K projection Mtile m over a 1024-token half hf.
                One LDWEIGHTS per contraction chunk (shared by both 512 MMs)."""
                lo = hf * NH
                w_sb, dst = (wq_sb, (qs if m == 0 else qs2)) if which == "q" else (
                    wk_sb, None
                )
                wid = min(NH, (NK - lo) if which == "k" else NH)
                if wid <= 0:
                    return
                t = sps.tile([128, NH], fp32, tag="s", name="qk_t")[:, :wid]
                for kk in range(DCH):
                    for n2 in range(0, wid, 512):
                        w2 = min(512, wid - n2)
                        nc.tensor.matmul(
                            t[:, n2 : n2 + w2],
                            lhsT=w_sb[:, kk, m * 128 : (m + 1) * 128],
                            rhs=xT_sb[:, kk, lo + n2 : lo + n2 + w2],
                            start=(kk == 0),
                            stop=(kk == DCH - 1),
                        )
                if which == "q":
                    nc.vector.tensor_copy(dst[:, lo : lo + wid], t[:])
                else:
                    nc.vector.tensor_copy(kst[:, m, lo : lo + wid], t[:])

            def v_pass(c):
                """V projection for key chunk c -> V3 (values * mf, ones col)."""
                v_t = sps.tile([128, NH], fp32, tag="s", name="v_t")[:, : HPG * HD]
                for kk in range(DCH):
                    nc.tensor.matmul(
                        v_t[:],
                        lhsT=xT_sb[:, kk, c * 128 : (c + 1) * 128],
                        rhs=wv_sb[:, kk, :],
                        start=(kk == 0),
                        stop=(kk == DCH - 1),
                    )
                nc.vector.tensor_scalar_mul(
                    V3[:, c, :, 0:HD],
                    v_t[:].rearrange("p (h d) -> p h d", h=HPG),
                    mf_sb[:, c : c + 1],
                )
                nc.vector.tensor_copy(
                    V3[:, c, :, HD], mf_sb[:, c : c + 1].to_broadcast((128, HPG))
                )

            OnA = [wpool.tile([128, NH], bfl, tag=f"OnA{q}", name=f"OnA{q}") for q in range(2)]
            OnB = [wpool.tile([64, NH], bfl, tag=f"OnB{q}", name=f"OnB{q}") for q in range(2)]

            def normalize(h, qh, o_t):
                """osb <- o (frees PSUM fast); rb = 1/rowsum bcast; OnX = o*rb."""
                osb = wpool.tile([HD + 1, NH], fp32, tag="osb", bufs=2, name="osb")
                nc.vector.tensor_copy(osb[:], o_t[:])
                sums = wpool.tile([1, NH], fp32, tag="sums", bufs=2, name="sums")
                nc.vector.tensor_copy(sums[:], osb[HD : HD + 1, :])
                rbb = wpool.tile([HD, NH], fp32, tag="rbb", bufs=2, name="rbb")
                nc.gpsimd.partition_broadcast(rbb[:], sums[:])
                rb = wpool.tile([HD, NH], fp32, tag="rb", bufs=2, name="rb")
                nc.vector.reciprocal_approx_fast(rb[:], rbb[:])
                dst = OnA[qh][h * 64 : (h + 1) * 64, :] if h < 2 else OnB[qh][:, :]
                nc.vector.tensor_tensor(dst, osb[0:HD, :], rb[:], MULT)

            def qk_mm(s_t, ksrc, qsrc, base, c, qh):
                for n2 in range(2):
                    nc.tensor.matmul(
                        s_t[:, n2 * 512 : (n2 + 1) * 512],
                        lhsT=ksrc[base : base + 64, c * 128 : (c + 1) * 128],
                        rhs=qsrc[
                            base : base + 64,
                            qh * NH + n2 * 512 : qh * NH + (n2 + 1) * 512,
                        ],
                        start=True,
                        stop=True,
                    )

            def pv_mm(o_t, p_t, c, h):
                for n2 in range(2):
                    nc.tensor.matmul(
                        o_t[:, n2 * 512 : (n2 + 1) * 512],
                        lhsT=V3[:, c, h, :],
                        rhs=p_t[:, c, n2 * 512 : (n2 + 1) * 512],
                        start=(c == 0),
                        stop=(c == nkch - 1),
                    )

            def pair_unit(qh, extras=()):
                """Heads 0+1, chunk-interleaved, query half qh. PV for chunk
                c-1 is emitted after chunk c's QK (FIFO queue stays unblocked).
                extras: thunks inserted one per chunk (fill-in projections)."""
                p_t = [
                    ppool.tile([128, nkch, NH], bfl, tag="p", name=f"pu{h}")
                    for h in range(2)
                ]
                o_t = [ops.tile([HD + 1, NH], fp32, tag="o", name=f"ou{h}") for h in range(2)]
                ex = list(extras)
                for c in range(nkch):
                    if c < len(ex):
                        ex[c]()
                    s_t = []
                    for h in range(2):
                        st = sps.tile([128, NH], fp32, tag="s", name=f"s{h}")
                        qk_mm(st, kst[:, 0], qs, 64 * h, c, qh)
                        s_t.append(st)
                    for h in range(2):
                        nc.scalar.activation(p_t[h][:, c, :], s_t[h][:], EXP, scale=SCALE)
                    if c > 0:
                        for h in range(2):
                            pv_mm(o_t[h], p_t[h], c - 1, h)
                for h in range(2):
                    pv_mm(o_t[h], p_t[h], nkch - 1, h)
                for h in range(2):
                    normalize(h, qh, o_t[h])

            def h2_unit(qh, extras=()):
                """Head 2 over query half qh; kst[:,1]/qs2 hold [k2|k2]/[q2|q2]
                so chunks alternate array row-halves."""
                p_t = ppool.tile([128, nkch, NH], bfl, tag="p", name="pu2")
                o_t = ops.tile([HD + 1, NH], fp32, tag="o", name="ou2")
                ex = list(extras)
                for c in range(nkch):
                    if c < len(ex):
                        ex[c]()
                    s_t = sps.tile([128, NH], fp32, tag="s", name="s2")
                    qk_mm(s_t, kst[:, 1], qs2, 64 * (c % 2), c, qh)
                    nc.scalar.activation(p_t[:, c, :], s_t[:], EXP, scale=SCALE)
                    if c > 0:
                        pv_mm(o_t, p_t, c - 1, 2)
                pv_mm(o_t, p_t, nkch - 1, 2)
                normalize(2, qh, o_t)

            ag_in = [
                nc.dram_tensor(f"ag_in{q}", [NCORES * HPG * HD, 128], bfl)
                for q in range(2)
            ]
            ag_out = [
                nc.dram_tensor(f"ag_out{q}", [NCORES * HPG * HD, 128], bfl)
                for q in range(2)
            ]

            def bounce(qh):
                agi_r = ag_in[qh].rearrange("(j p) t -> p j t", j=NCORES)
                nc.sync.dma_start(
                    agi_r[0:128, :, :],
                    OnA[qh][:, :].rearrange("p (j t) -> p j t", j=NCORES),
                )
                nc.sync.dma_start(
                    agi_r[128:192, :, :],
                    OnB[qh][:, :].rearrange("p (j t) -> p j t", j=NCORES),
                )

            def a2a(qh):
                nc.gpsimd.collective_compute(
                    "AllToAll",
                    mybir.AluOpType.bypass,
                    replica_groups=[[0, 1, 2, 3, 4, 5, 6, 7]],
                    ins=[ag_in[qh][:].opt()],
                    outs=[ag_out[qh][:].opt()],
                )

            out_r = out.rearrange("(o p) t -> p o t", p=128)

            def proj_dma(qh):
                at_sb = wpool.tile(
                    [128, 2, DCH, 128], bfl, tag="at", bufs=2, name="at_sb"
                )
                nc.sync.dma_start(
                    at_sb[:], ag_out[qh].rearrange("(b o p) t -> p b o t", p=128, b=2)
                )
                return at_sb

            def proj_pass(qh, at_sb):
                for m in range(DCH):
                    y_ps = sps.tile([128, NH], fp32, tag="s", name="y_ps")[:, :256]
                    for kk in range(DCH):
                        nc.tensor.matmul(
                            y_ps[:].rearrange("p (b t) -> p b t", b=2),
                            lhsT=wp_sb[:, kk, m * 128 : (m + 1) * 128],
                            rhs=at_sb[:, :, kk, :],
                            start=(kk == 0),
                            stop=(kk == DCH - 1),
                        )
                    y_sb = wpool.tile([128, 256], fp32, tag="y", bufs=2, name="y_sb")
                    nc.vector.tensor_scalar_add(y_sb[:], y_ps[:], bp_sb[:, m : m + 1])
                    nc.sync.dma_start(
                        out_r[:, m, qh * 256 : (qh + 1) * 256], y_sb[:]
                    )

            def warm_fill(n):
                """Junk matmuls that keep the PE HAM-warm while waiting."""
                for _ in range(n):
                    w_ps = sps.tile([128, NH], fp32, tag="s", name="w_ps")[:, :512]
                    nc.tensor.matmul(
                        w_ps[:], lhsT=wp_sb[:, 0, 0:128], rhs=wp_sb[:, 1, 0:512],
                        start=True, stop=True,
                    )

            # ---------------- schedule ----------------
            # bootstrap: k0|k1 over all keys, q0|q1 and q2|q2 over half-0
            # tokens, V3(0,1) -> attention can start ~as soon as DMAs land.
            khf = 1 + (1 if NK > NH else 0)
            for hf in range(khf):
                qk_pass("k", 0, hf)
            qk_pass("q", 0, 0)
            qk_pass("q", 1, 0)
            v_pass(0)
            if nkch > 1:
                v_pass(1)

            # fill-in thunks: v(c) must be inserted at slot <= c.
            k1 = [lambda hf=hf: qk_pass("k", 1, hf) for hf in range(khf)]
            vs = [lambda c=c: v_pass(c) for c in range(2, nkch)]
            ex_p0 = [k1[0]] + vs + k1[1:2]
            ex_h0 = k1[2:] + [lambda: qk_pass("q", 0, 1)]
            ex_p1 = [lambda: qk_pass("q", 1, 1)]

            # ---- half 0 (tokens 0-1023)
            pair_unit(0, extras=ex_p0[:nkch])
            for f in ex_p0[nkch:]:
                f()
            h2_unit(0, extras=ex_h0[:nkch])
            for f in ex_h0[nkch:]:
                f()
            bounce(0)
            a2a(0)

            # ---- half 1 (tokens 1024-2047); A2A#0 + proj#0 hide under it
            pair_unit(1, extras=ex_p1)
            h2_unit(1)
            at0 = proj_dma(0)
            bounce(1)
            a2a(1)
            proj_pass(0, at0)
            warm_fill(24)
            at1 = proj_dma(1)
            proj_pass(1, at1)

            ops_cm.__exit__(None, None, None)
            sps_cm.__exit__(None, None, None)

    nc.finalize()
    return nc


def _prep(x, mask, w_qkv, w_proj, b_proj):
    """Host-side compaction: per-batch token permutation (kept keys first) and
    per-core input shards."""
    perms, counts = [], []
    for b in range(B):
        perm = np.argsort(1 - mask[b], kind="stable")
        perms.append(perm)
        counts.append(int(mask[b].sum()))
    nkch = max(1, int(np.ceil(max(counts) / 128)))
    NK = nkch * 128

    xTs = []
    mfs = []
    for b in range(B):
        xp = np.ascontiguousarray(x[b][perms[b]].T).astype(bf)
        xTs.append(xp)
        m = np.zeros(NK, dtype=np.float32)
        m[: counts[b]] = 1.0
        mfs.append(np.ascontiguousarray(m.reshape(nkch, 128).T))

    bp_t = np.ascontiguousarray(b_proj.astype(np.float32).reshape(DCH, 128).T)
    wp_t = w_proj.astype(bf)

    in_maps = []
    for c in range(NCORES):
        b, g = c // GPB, c % GPB
        heads = [3 * g, 3 * g + 1, 3 * g + 2]
        q_cols = [h * HD + d for h in (heads[0], heads[1], heads[2], heads[2]) for d in range(HD)]
        k_cols = [C + h * HD + d for h in (heads[0], heads[1], heads[2], heads[2]) for d in range(HD)]
        v_cols = [2 * C + h * HD + d for h in heads for d in range(HD)]
        in_maps.append(
            {
                "xT": xTs[b],
                "wq": np.ascontiguousarray(w_qkv[:, q_cols]).astype(bf),
                "wk": np.ascontiguousarray(w_qkv[:, k_cols]).astype(bf),
                "wv": np.ascontiguousarray(w_qkv[:, v_cols]).astype(bf),
                "wp": wp_t,
                "bp": bp_t,
                "mf": mfs[b],
            }
        )
    return in_maps, perms, nkch


def kernel(x, mask, w_qkv, w_proj, b_proj, _trace=False):
    from concourse.bass_utils import run_bass_kernel_spmd

    x = np.asarray(x, dtype=np.float32)
    mask = np.asarray(mask)
    w_qkv = np.asarray(w_qkv, dtype=np.float32)
    w_proj = np.asarray(w_proj, dtype=np.float32)
    b_proj = np.asarray(b_proj, dtype=np.float32)
    in_maps, perms, nkch = _prep(x, mask, w_qkv, w_proj, b_proj)
    if ("nc", nkch) not in _cache:
        _cache[("nc", nkch)] = _build(nkch)
    nc = _cache[("nc", nkch)]
    res = run_bass_kernel_spmd(nc, in_maps, core_ids=list(range(NCORES)), trace=_trace)
    y = np.empty((B, N, C), dtype=np.float32)
    for c in range(NCORES):
        o = np.asarray(res.results[c]["out"])
        for qh in range(2):
            base = qh * NH + c * 128
            for b in range(B):
                y[b, perms[b][base : base + 128]] = o[
                    :, qh * 256 + b * 128 : qh * 256 + (b + 1) * 128
                ].T
    if _trace:
        _cache["last_exec_time_ns"] = res.exec_time_ns
        _cache["last_profile"] = res.profile_json
    return y
